# revision 11
# baseline (speedup 1.0000x reference)
"""Trainium2 Bass kernel for nn_EqualizedConv2dModulated.

Reference math (per sample b):
    W' = weight * WS,  WS = 1/sqrt(Cin*KH*KW)
    Wm[b] = s[b,ci] * W'                       (modulation)
    sigma[b,co] = sqrt(sum_{ci,k} Wm^2 + 1e-8) (demodulation)
    out[b] = conv2d_same(x[b], Wm[b]/sigma[b])

Because conv is linear in the weight, fold the per-sample modulation into
the activations and the demodulation into the output:
    out[b,co] = invs[b,co] * conv2d_same(x[b] * s[b,:], weight)[co]
    invs[b,co] = 1/sqrt(T[b,co] + 1e-8/WS^2),  T = sum_{ci,k} s^2 * W^2
(the WEIGHT_SCALE constant cancels exactly).

Sharding: data-parallel over batch, 2 samples per core on 8 cores.
Weights are host-transposed to [tap, ci, co] (layout only) and replicated.
All matmuls run in float32r (tf32-like: ~1.5e-4 rel err, ~bf16 speed).

Per core:
  - x[b] is scaled by s[b,ci] (DVE per-partition scalar mul, f32->f32r)
    into a zero-padded [ci, 34, 34] image.
  - conv = 9 taps x 4 ci-chunks accumulating matmuls per (co-chunk,
    16-row pixel block): lhsT = W_tap[ci,co] (stationary), rhs = shifted
    window of the padded image, PSUM [co,512] f32.
  - T[b,co] via matmul: lhsT = (s^2)T [ci,2] (stationary), rhs = W_tap^2
    [ci,co], accumulated over all 36 (tap,ci-chunk) into PSUM [2,512].
  - invs = 1/sqrt(T + eps'); transposed to [co,b] via a tiny DRAM
    round-trip; applied as the PSUM->SBUF copy scale on ScalarE.
"""

import sys
import types

import numpy as np

import bass_rust
import concourse.bass as bass
import concourse.mybir as mybir
import concourse.tile as tile_mod
import concourse.bass_utils as bass_utils
from concourse.tile import TileContext, ScopedClock
from concourse.bass_utils import run_bass_kernel_spmd

N_CORES = 8
B, CIN, H, W = 16, 512, 32, 32
COUT, KH, KW = 512, 3, 3
PER_CORE = B // N_CORES  # 2 samples per core
KC = CIN // 128  # ci chunks
MC = COUT // 128  # co chunks
NP = 2  # pixel blocks of 16 rows (512 px) each
TAPS = [(dy, dx) for dy in range(3) for dx in range(3)]
EPS_FOLDED = 1e-8 * (CIN * KH * KW)  # 1e-8 / WEIGHT_SCALE^2

F32 = mybir.dt.float32
F32R = mybir.dt.float32r

# set by test harnesses; kernel() reads them
TRACE = False
LAST_EXEC_NS = None
LAST_TRACE = None


def _patched_drain_and_barrier(self, tick_clock, wait_clock):
    """Walrus in this container rejects >1 sync wait per instruction; split
    the TileContext exit drain's waits across extra SP nops."""
    nc = self.nc
    drain_inst = nc.sync.drain()
    wait_clock.add_sem_waits(
        drain_inst.ins, ScopedClock({None: tick_clock.global_clock})
    )
    si = drain_inst.ins.sync_info
    waits = list(si.on_wait or [])
    if len(waits) > 1:
        si.on_wait = waits[:1]
        for w in waits[1:]:
            nop = nc.sync.nop(nofuse=True, hint="drain_split")
            nop.ins.sync_info = bass_rust.SyncInfo(on_wait=[w], on_update=[])
    nc.all_engine_barrier()
    assert self.sems is not None
    popped = nc._tile_sem_poison_stack.pop()
    assert popped is self._sem_poison
    nc.clear_and_free_semaphores(list(self.sems.allocated().values()))
    nc.all_engine_barrier()


def _split_multi_waits(nc, max_waits=1):
    """Hoist extra sync waits onto same-engine NoOps inserted directly before
    the owning instruction (engine streams are in-order, so gating semantics
    are identical). Needed because this walrus build allows only one sync
    wait per instruction."""
    counter = 0
    for f in nc.m.functions:
        for bb in f.blocks:
            insts = list(bb.instructions)
            out = []
            changed = False
            for inst in insts:
                si = inst.sync_info
                waits = list(si.on_wait) if (si and si.on_wait) else []
                if len(waits) > max_waits:
                    keep = waits[:max_waits]
                    extra = waits[max_waits:]
                    for j in range(0, len(extra), max_waits):
                        nop = bass_rust.InstNoOp(
                            name=f"I-waitsplit-{counter}", ins=[], outs=[]
                        )
                        counter += 1
                        nop.engine = inst.engine
                        nop.sync_info = bass_rust.SyncInfo(
                            on_wait=extra[j : j + max_waits], on_update=[]
                        )
                        nc.register_instruction(nop)
                        out.append(nop)
                    si.on_wait = keep
                    changed = True
                out.append(inst)
            if changed:
                bb.instructions = out


_orig_run_command = bass_utils.run_command


def _run_command_ldwopt(argv, **kwargs):
    argv = [a.replace("--enable-ldw-opt=false", "--enable-ldw-opt=true") for a in argv]
    return _orig_run_command(argv, **kwargs)


def _install_patches():
    tile_mod.TileContext._drain_and_barrier = _patched_drain_and_barrier
    bass_utils.run_command = _run_command_ldwopt
    if TRACE and "antenv.axon_hooks" not in sys.modules:
        try:
            from trn_agent_boot.trn_boot import _ntff_profile_via_ctypes

            hook = _ntff_profile_via_ctypes("/opt/axon/libaxon_pjrt.so")
            mod = types.ModuleType("antenv.axon_hooks")
            mod.get_axon_ntff_profile_hook = lambda: hook
            mod.set_axon_ntff_profile_hook = lambda h: None
            sys.modules["antenv.axon_hooks"] = mod
            bass_utils.upload_artifacts = lambda tmpdir: tmpdir
        except Exception:
            pass


def _build_program():
    nc = bass.Bass("TRN2", target_bir_lowering=False, debug=False, num_devices=N_CORES)
    xd = nc.declare_dram_parameter("x", [PER_CORE, CIN, H, W], F32, isOutput=False)
    sd = nc.declare_dram_parameter("s", [PER_CORE, CIN], F32, isOutput=False)
    wtd = nc.declare_dram_parameter("wt", [9, CIN, COUT], F32R, isOutput=False)
    od = nc.declare_dram_parameter("o", [PER_CORE, COUT, H, W], F32, isOutput=True)
    sig_scr = nc.dram_tensor("sig_scr", [PER_CORE, COUT], F32)

    with TileContext(nc) as tc:
        with (
            tc.tile_pool(name="wpool", bufs=1) as wpool,
            tc.tile_pool(name="xpadp", bufs=1) as xpadp,
            tc.tile_pool(name="xstage", bufs=4) as xstage,
            tc.tile_pool(name="small", bufs=1) as small,
            tc.tile_pool(name="sqpool", bufs=2) as sqpool,
            tc.tile_pool(name="opool", bufs=6) as opool,
            tc.tile_pool(name="psum", bufs=7, space="PSUM") as psum_pool,
            tc.tile_pool(name="psumS", bufs=1, space="PSUM") as psumS_pool,
        ):
            # --- s: load transposed [ci, b], square to f32r ---
            sT = small.tile([128, KC, PER_CORE], F32)
            sdT = sd.rearrange("b c -> c b")
            for kc in range(KC):
                nc.gpsimd.dma_start(
                    out=sT[:, kc], in_=sdT[kc * 128 : (kc + 1) * 128]
                )
            s2T = small.tile([128, KC, PER_CORE], F32R)
            for kc in range(KC):
                nc.vector.tensor_mul(s2T[:, kc], sT[:, kc], sT[:, kc])

            # --- x: load, modulate by s, write into zero-padded f32r image ---
            # (memset does not support f32r: zero borders via DVE cast-copies
            # from a small f32 zero tile; interior is fully overwritten)
            zsrc = small.tile([128, H + 2], F32)
            nc.vector.memset(zsrc, 0.0)
            zcol = zsrc.rearrange("p (a b) -> p a b", b=1)
            xpads = []
            for smp in range(PER_CORE):
                xp = xpadp.tile(
                    [128, KC, H + 2, W + 2], F32R, tag=f"xpad{smp}", name=f"xpad{smp}"
                )
                for kc in range(KC):
                    nc.vector.tensor_copy(xp[:, kc, 0, :], zsrc)
                    nc.vector.tensor_copy(xp[:, kc, H + 1, :], zsrc)
                    nc.vector.tensor_copy(xp[:, kc, :, 0:1], zcol)
                    nc.vector.tensor_copy(xp[:, kc, :, W + 1 : W + 2], zcol)
                xpads.append(xp)
            for kc in range(KC):
                for smp in range(PER_CORE):
                    xs = xstage.tile([128, H, W], F32, tag="xs", name=f"xs{smp}_{kc}")
                    eng = nc.sync if smp == 0 else nc.scalar
                    eng.dma_start(out=xs, in_=xd[smp, kc * 128 : (kc + 1) * 128])
                    nc.vector.tensor_scalar_mul(
                        xpads[smp][:, kc, 1 : H + 1, 1 : W + 1],
                        xs,
                        sT[:, kc, smp : smp + 1],
                    )

            # --- weights: cast-DMA f32 -> f32r, [tap][ci-chunk][128, co 512] ---
            wt_tiles = []
            for t in range(9):
                wt_t = wpool.tile(
                    [128, KC, 512], F32R, tag=f"wt{t}", name=f"wt{t}"
                )
                eng = nc.sync if t % 2 == 0 else nc.scalar
                eng.dma_start(
                    out=wt_t, in_=wtd[t].rearrange("(c p) co -> p c co", p=128)
                )
                wt_tiles.append(wt_t)

            def wslice(t, kc):
                return wt_tiles[t][:, kc]

            # --- sigma: T[b,co] = sum s^2 W^2 via matmul (s2T stationary) ---
            psumS = psumS_pool.tile([PER_CORE, 512], F32)
            for i, (t, kc) in enumerate([(t, kc) for t in range(9) for kc in range(KC)]):
                sq = sqpool.tile([128, 512], F32R, tag="sq", name=f"sq{t}_{kc}")
                nc.vector.tensor_mul(sq, wslice(t, kc), wslice(t, kc))
                nc.tensor.matmul(
                    psumS,
                    s2T[:, kc],
                    sq,
                    start=(i == 0),
                    stop=(i == 9 * KC - 1),
                )
            epsT = small.tile([PER_CORE, 1], F32)
            nc.vector.memset(epsT, float(EPS_FOLDED))
            sig = small.tile([PER_CORE, 512], F32)
            nc.scalar.activation(
                out=sig,
                in_=psumS,
                func=mybir.ActivationFunctionType.Sqrt,
                bias=epsT,
                scale=1.0,
            )
            isig = small.tile([PER_CORE, 512], F32)
            nc.vector.reciprocal(out=isig, in_=sig)
            # transpose [b, co] -> [co, b] via DRAM round-trip (tiny)
            nc.gpsimd.dma_start(out=sig_scr[:], in_=isig)
            isigT = small.tile([128, MC, PER_CORE], F32)
            scrT = sig_scr.ap().rearrange("b c -> c b")
            for mc in range(MC):
                nc.gpsimd.dma_start(
                    out=isigT[:, mc], in_=scrT[mc * 128 : (mc + 1) * 128]
                )

            # --- conv: 36 accumulating matmuls per (mc, sample, pixel block) ---
            for mc in range(MC):
                psums = {}
                for smp in range(PER_CORE):
                    for p in range(NP):
                        psums[(smp, p)] = psum_pool.tile(
                            [128, 512], F32, tag="ps", name=f"ps{mc}_{smp}_{p}"
                        )
                for i, ((dy, dx), kc) in enumerate(
                    [(tap, kc) for tap in TAPS for kc in range(KC)]
                ):
                    t = TAPS.index((dy, dx))
                    lhsT = wslice(t, kc)[:, mc * 128 : (mc + 1) * 128]
                    for smp in range(PER_CORE):
                        for p in range(NP):
                            r0 = p * 16
                            rhs = xpads[smp][
                                :, kc, r0 + dy : r0 + dy + 16, dx : dx + 32
                            ]
                            nc.tensor.matmul(
                                psums[(smp, p)],
                                lhsT,
                                rhs,
                                start=(i == 0),
                                stop=(i == 9 * KC - 1),
                            )
                for smp in range(PER_CORE):
                    for p in range(NP):
                        ot = opool.tile(
                            [128, 16, W], F32, tag="ot", name=f"ot{mc}_{smp}_{p}"
                        )
                        nc.scalar.activation(
                            out=ot,
                            in_=psums[(smp, p)].rearrange("q (h w) -> q h w", w=W),
                            func=mybir.ActivationFunctionType.Copy,
                            scale=isigT[:, mc, smp : smp + 1],
                        )
                        nc.gpsimd.dma_start(
                            out=od[
                                smp,
                                mc * 128 : (mc + 1) * 128,
                                p * 16 : (p + 1) * 16,
                                :,
                            ],
                            in_=ot,
                        )

    _split_multi_waits(nc)
    return nc


_PROGRAM_CACHE = {}


def kernel(x, s, weight):
    global LAST_EXEC_NS, LAST_TRACE
    _install_patches()
    if "nc" not in _PROGRAM_CACHE:
        _PROGRAM_CACHE["nc"] = _build_program()
    nc = _PROGRAM_CACHE["nc"]

    x = np.ascontiguousarray(x, dtype=np.float32)
    s = np.ascontiguousarray(s, dtype=np.float32)
    weight = np.ascontiguousarray(weight, dtype=np.float32)
    # host layout prep only: [co, ci, kh, kw] -> [kh*kw, ci, co]
    wt = np.ascontiguousarray(weight.transpose(2, 3, 1, 0).reshape(9, CIN, COUT))

    in_maps = [
        {
            "x": x[i * PER_CORE : (i + 1) * PER_CORE],
            "s": s[i * PER_CORE : (i + 1) * PER_CORE],
            "wt": wt,
        }
        for i in range(N_CORES)
    ]
    res = run_bass_kernel_spmd(nc, in_maps, list(range(N_CORES)), trace=TRACE)
    LAST_EXEC_NS = res.exec_time_ns
    LAST_TRACE = res.instructions_and_trace[1] if res.instructions_and_trace else None
    out = np.concatenate([res.results[i]["o"] for i in range(N_CORES)], axis=0)
    return out
